# revision 40
# baseline (speedup 1.0000x reference)
"""MultiHeadSelfAttention2D Trainium2 kernel (8-core SPMD, full I/O).

Problem: B=4, C_IN=C_OUT=256, HEADS=8, H=W=48 (m = 2304), fp32.
  vh, zh, qh = per-head 1x1-conv projections of x; rh = fixed 2D sin/cos PE.
  scores = vh^T (zh + rh)  (per b,h); attn = softmax(scores/sqrt(dh), axis=n)
  out = attn @ qh^T  -> (b, c_out, h, w)

Sharding: core = 2*b + head_half. Each core handles one batch image and 4
heads (=128 output channels). No cross-core communication.

Per-core pipeline (bf16 matmul path, fp32 psum accumulation; layouts chosen
so no on-chip transposes are needed):
  - x / weights ship from host ALREADY in bf16 (halves input DMA, removes
    all on-chip casts: the casts cost ~6.5us ACT + ~3us DVE in v1)
  - vh2/kz2 [64, 2*2304] bf16: head h at row-block 32*(h%2), col-block h//2.
    Two row blocks let two scores matmuls run concurrently on disjoint PE
    row-groups while writing different psum banks (same-bank concurrent
    sub-array writes are fatal on TRN2).
  - qhT[n, ch] bf16, nb-major [128, 18*128]
  - attention runs as ONE flat software-pipelined loop over g = (m-chunk,
    n-block): scores(g+2) | exp(g+1) emitted every step; PV(g-2), PV(g-1),
    sums(g-2) batch on ODD steps only, so the PE stream is
    [SC][SC, PV, PV, SUM] per 2 steps.  The deep PV lag keeps the in-order
    PE queue from ever reaching a PV matmul before its exp block, and the
    2-step batching halves the row<->col tiling-config switches (each
    ~120ns: cross-mode LDWEIGHTS cannot overlap running streams).  Per g:
      scoresT[n,m] 4 matmuls (2-way row-concurrent) -> psc [128, 1024]
      -> exp split by COLUMNS every block: ACT does ~680 cols (exact Exp),
         DVE does ~340 via the Schraudolph bit trick (one tensor_scalar:
         bf16_bits = int16(psc*A + B), ~+-3%).  The DVE window ROTATES over
         3 positions with g%3 (a fixed window makes its m-columns 100%
         Schraudolph -> 1.5e-3 abs err, over budget; rotation dilutes to
         1/3 per output column like the passing block-alternating scheme).
         When DVE gets the middle piece, ACT covers the two outer pieces
         with ONE 2-range strided AP (stride 680 x 2, inner 344).
      -> PV col-tiled 4-head matmuls (fully concurrent); row-sums via
         ones-matmul every OTHER n-block on a DVE-precomputed et(j)+et(j+1)
         pair (halves PE's sums matmuls)
  - projections are emitted chunk-major and interleaved into the first
    n-blocks of the attention loop (chunk c gates scores per CHUNK_FOR_J
    with ~2 n-blocks of lookahead), k=0/k=1 DMA halves adjacent
  - DVE reciprocal_approx_fast + multiply for the normalization, DMA out

Measured: 202.5us (prior session) -> 179.8us; rel err 1.756e-2 (gate 2e-2).
Steady state ~873ns/g: PE array ~98% busy (the pacer), DVE 91%, ACT 85%.
Dead ends (all MEASURED this session or earlier, do not retry):
zero-weight filler matmuls serialize (+90-100us); 512-wide score windows /
4-way scores / MCH=512 all need >=9 psum banks (8 exist); fp8 DoubleRow
gains nothing (matmuls here are rhs-column-stream-bound); ANY gpsimd
tensor_add in the loop (+3us as alternate pair-adds at 2-step lag, +34us
as level-2 quad adds at 4-step lag); 4-step PE batching drops array
occupancy to 94%; 4-way proj col-groups + SBUF relocation of v (+5-13us
net, compresses the proj phase and loses the HAM-warm window);
block-alternating exp 12:6 (bimodal 873/1114ns pace -- the 3-deep psc
pool cannot buffer across the alternation); fixed-window column-split exp
(4.4e-2 rel err).  HAM: PE un-throttles to 2.4 GHz only at ~88%+
STREAM-cycle duty over 3.4us; the loop sits at ~85% -> 1.2 GHz forever.
"""

import numpy as np
from contextlib import ExitStack

import ml_dtypes

import concourse.bass as bass
import concourse.bacc as bacc
import concourse.tile as tile
from concourse import mybir
from concourse.bass_utils import run_bass_kernel_spmd

F32 = mybir.dt.float32
BF16 = mybir.dt.bfloat16
NP_BF16 = ml_dtypes.bfloat16

B, C_IN, C_OUT, HEADS, H, W = 4, 256, 256, 8, 48, 48
M = H * W  # 2304
DH = C_OUT // HEADS  # 32
HPC = 4  # heads per core
CH = HPC * DH  # 128 channels per core
NB = M // 128  # 18 n-blocks of 128
MCH = 256  # m-chunk width
NMCH = M // MCH  # 9
INV_SCALE = float(1.0 / np.sqrt(np.float32(DH)))  # softmax temperature

# ACT (exact exp) and DVE (Schraudolph int16 bit-trick exp) split EVERY
# exp block by columns.  Alternating whole blocks between the engines (v2)
# made the per-g pace bimodal -- ACT-blocks ran at ACT's 1114ns, DVE-blocks
# at the PE's 873ns, averaging 1034 -- because the 3-deep psc pool cannot
# buffer enough slack to amortize the alternation.  A per-block column
# split equalizes every g at ~880ns on all three engines.
#
# CRITICAL: the DVE window must ROTATE across n-blocks (g%3).  The et
# columns are the m axis; a FIXED window makes its output columns 100%
# Schraudolph (no dilution across the softmax's n-sum) -- measured abs err
# 1.5e-3, over the 7e-4 budget.  Rotating over 3 positions gives every
# output column exactly 1/3 Schraudolph n-contributions, same dilution as
# the passing v2/v3 configs (5.7e-4).  When the rotation puts DVE in the
# middle, ACT covers {[0,344) u [680,1024)} with ONE 2-range strided AP
# (stride 680 x 2, inner 344) -- two ACTIVATEs would cost +293ns/g.
EXP_SPLITS = [
    # (dve_lo, dve_hi, act is 2-range?)
    (0, 344, False),     # ACT [344, 1024)
    (344, 680, True),    # ACT [0,344) u [680,1024)
    (680, 1024, False),  # ACT [0, 680)
]
SCHRAUD_A = float(128.0 / np.log(2.0) * INV_SCALE)
# 127*128 (bf16 bias) - 5.51 (minimax centering) + 0.5 (trunc->round)
SCHRAUD_B = 16251.0

PROJ_CHUNKS = [(0, 512), (512, 512), (1024, 512), (1536, 512), (2048, 256)]
# chunk index that covers scores n-block j (kz2 cols j*128)
CHUNK_FOR_J = [
    next(ci for ci, (off, wd) in enumerate(PROJ_CHUNKS) if off <= j * 128 < off + wd)
    for j in range(NB)
]

# scores psum / exp-tile column block per head: two-way row concurrency,
# h0/h2 share psum bank 0 (row block 0), h1/h3 share bank 1 (row block 32)
ECOL = [0, 2, 1, 3]  # head -> 256-col block in psc/et


def _kernel_body(ctx: ExitStack, tc: tile.TileContext, xb_d, wb_d, rh_d, out_d):
    nc = tc.nc

    consts = ctx.enter_context(tc.tile_pool(name="consts", bufs=1))
    expp = ctx.enter_context(tc.tile_pool(name="expp", bufs=6))
    sump = ctx.enter_context(tc.tile_pool(name="sump", bufs=3))
    outp = ctx.enter_context(tc.tile_pool(name="outp", bufs=3))
    psum_s = ctx.enter_context(tc.tile_pool(name="psum_s", bufs=3, space="PSUM"))
    psum_o = ctx.enter_context(tc.tile_pool(name="psum_o", bufs=2, space="PSUM"))

    # ---- persistent SBUF tensors ----
    rh_sb = consts.tile([64, 2 * M], F32)
    x_bf = consts.tile([128, 2, M], BF16)
    wb_sb = consts.tile([128, 3, 2, CH], BF16)
    vh2 = consts.tile([64, 2 * M], BF16)  # [32*(h%2)+c, (h//2)*M + m]
    kz2 = consts.tile([64, 2 * M], BF16)
    qt_sb = consts.tile([128, M], BF16)   # [n, ch] nb-major blocks
    ones_sb = consts.tile([128, DH], BF16)

    wv_bf = wb_sb[:, 0]
    wz_bf = wb_sb[:, 1]
    wq_bf = wb_sb[:, 2]

    # wv/wz land first (wq is only needed by the qt projections, a few us
    # later), then x/rh interleaved per chunk with both contraction halves
    # (k=0,1) adjacent, so projection chunk c can start as soon as its slice
    # arrives instead of after most of the input.
    # one 2-range DMA per chunk per tensor (both k/pair halves via a strided
    # AP): each DMA_DIRECT2D costs ~0.6us of serial Sync-queue issue time,
    # and the queue is the early-phase serialization point
    def two_halves(ap2d, off, wd):
        base = ap2d[:, off:off + wd]
        dims = list(base.ap)
        return bass.AP(base.tensor, base.offset, [dims[0], [M, 2], [1, wd]])

    nc.sync.dma_start(out=wb_sb[:, 0:2], in_=wb_d.ap()[:, 0:2])
    for ci, (off, wd) in enumerate(PROJ_CHUNKS):
        nc.sync.dma_start(
            out=x_bf[:, :, off:off + wd], in_=two_halves(xb_d.ap(), off, wd)
        )
        if ci == 0:
            nc.sync.dma_start(out=wb_sb[:, 2:3], in_=wb_d.ap()[:, 2:3])
        nc.sync.dma_start(
            out=two_halves(rh_sb, off, wd), in_=two_halves(rh_d.ap(), off, wd)
        )
    nc.vector.memset(ones_sb, 1.0)

    # zero bias for Exp (one-time; cross-engine edge is paid once)

    # ---- projections (chunk-major; emission interleaved with attention) ----
    # 4-way col-group concurrency: z -> rows 32*(h%2) (q0/q32, in place for
    # kz2), v -> rows 64+32*(h%2) (q64/q96, staged then DMA'd down).  Both
    # pairs go in ONE psum tile (pair = bank); 4 concurrent streams per bank
    # at disjoint partitions are legal (only same-partition same-bank
    # concurrent writes are fatal).  Halves the vh/kz PE time vs the 2-way
    # v1 layout (z serialized behind v on the same col-groups there).
    def emit_proj_chunk(off, wd):
        # 2-way: v/z for heads (2*pair, 2*pair+1) at rows 32*(h%2); v in
        # bank 0, z in bank 1.  (A 4-way variant with z/v split across row
        # halves measured SLOWER overall: the needed SBUF->SBUF relocation
        # of v and the compressed proj phase cost more than the PE saved.)
        for pair in range(2):
            ps = psum_s.tile([128, 4 * MCH], F32, tag="psc", name="ps")
            for hh in range(2):  # head = 2*pair + hh
                h = 2 * pair + hh
                for k in range(2):
                    nc.tensor.matmul(
                        ps[32 * hh:32 * hh + 32, :wd],
                        lhsT=wv_bf[:, k, 32 * h:32 * h + 32],
                        rhs=x_bf[:, k, off:off + wd],
                        start=(k == 0),
                        stop=(k == 1),
                        tile_position=(0, 32 * hh),
                        skip_group_check=True,
                    )
                    nc.tensor.matmul(
                        ps[32 * hh:32 * hh + 32, 512:512 + wd],
                        lhsT=wz_bf[:, k, 32 * h:32 * h + 32],
                        rhs=x_bf[:, k, off:off + wd],
                        start=(k == 0),
                        stop=(k == 1),
                        tile_position=(0, 32 * hh),
                        skip_group_check=True,
                    )
            # vh2 copy on ACT (idle during the ramp) so DVE can focus on the
            # kz2 adds + early exps; kz2 gates the first scores matmuls
            nc.scalar.copy(
                out=vh2[:, pair * M + off: pair * M + off + wd], in_=ps[0:64, :wd]
            )
            nc.vector.tensor_add(
                out=kz2[:, pair * M + off: pair * M + off + wd],
                in0=ps[0:64, 512:512 + wd],
                in1=rh_sb[:, pair * M + off: pair * M + off + wd],
            )
        emit_proj_qt(off, wd)

    def emit_proj_qt(off, wd):
        # qhT for the n-blocks covered by this chunk, all in ONE psum tile
        # and one copy: fewer psum-pool rotations against the attention psc
        # stream
        nbs = list(range(off // 128, (off + wd) // 128))
        ps = psum_s.tile([128, 4 * MCH], F32, tag="psc", name="ps")
        for i, nb in enumerate(nbs):
            for k in range(2):
                nc.tensor.matmul(
                    ps[:, i * CH:(i + 1) * CH],
                    lhsT=x_bf[:, k, nb * 128:(nb + 1) * 128],
                    rhs=wq_bf[:, k, :],
                    start=(k == 0),
                    stop=(k == 1),
                )
        nc.vector.tensor_copy(
            out=qt_sb[:, nbs[0] * 128:(nbs[-1] + 1) * 128],
            in_=ps[:, 0:len(nbs) * CH],
        )

    # ---- attention ----
    def emit_scores(psc, j, m0):
        # h0 (rows 0-31 -> bank0) || h1 (rows 32-63 -> bank1) concurrent;
        # h2/h3 reuse the same PE rows so they serialize behind h0/h1.
        for h in (0, 1, 2, 3):
            rb = 32 * (h % 2)
            cb = (h // 2) * M
            nc.tensor.matmul(
                psc[:, ECOL[h] * MCH:(ECOL[h] + 1) * MCH],
                lhsT=kz2[rb:rb + 32, cb + j * 128: cb + (j + 1) * 128],
                rhs=vh2[rb:rb + 32, cb + m0: cb + m0 + MCH],
                start=True,
                stop=True,
                tile_position=(rb, 0),
                skip_group_check=True,
            )

    def emit_pv(po, j, et):
        # PV: out_unnorm[32h+d, m] += sum_n qhT[n, 32h+d] * expT_h[n, m]
        for h in range(HPC):
            nc.tensor.matmul(
                po[32 * h:32 * h + 32, 0:MCH],
                lhsT=qt_sb[:, j * 128 + 32 * h: j * 128 + 32 * h + 32],
                rhs=et[:, ECOL[h] * MCH:(ECOL[h] + 1) * MCH],
                start=(j == 0),
                stop=False,
                tile_position=(0, 32 * h),
                skip_group_check=True,
            )

    def emit_sums(po, etp, last):
        # row-sums of an et(j)+et(j+1) pair, replicated over each head slot
        for h in range(HPC):
            nc.tensor.matmul(
                po[32 * h:32 * h + 32, MCH:2 * MCH],
                lhsT=ones_sb,
                rhs=etp[:, ECOL[h] * MCH:(ECOL[h] + 1) * MCH],
                start=False,
                stop=last,
                tile_position=(0, 32 * h),
                skip_group_check=True,
            )

    # Flat software-pipelined loop over g = mc*NB + j. Scores run SLAG steps
    # ahead of exp, which runs one ahead of pair-adds, which run one ahead of
    # PV/sums -- the extra PV lag keeps the exp latency entirely off the PE
    # queue's critical path.
    SLAG = 2  # scores stream leads the exp stream by this many steps
    G = NMCH * NB
    pscs, ets, pos = {}, {}, {}

    def emit_scores_g(g):
        if not (0 <= g < G):
            return
        mc, j = divmod(g, NB)
        pscs[g] = psum_s.tile([128, 4 * MCH], F32, tag="psc", name="psc")
        emit_scores(pscs[g], j, mc * MCH)

    def two_range(ap_full):
        # {[0,344) u [680,1024)} as one AP: {stride 680 x 2} x {1 x 344}
        base = ap_full[:, 0:4 * MCH]
        dims = list(base.ap)
        return bass.AP(base.tensor, base.offset, [dims[0], [680, 2], [1, 344]])

    def emit_exp_g(g):
        if not (0 <= g < G):
            return
        psc = pscs[g]
        et = expp.tile([128, 4 * MCH], BF16, tag="et")
        dve_lo, dve_hi, act_split = EXP_SPLITS[g % 3]
        if act_split:
            act_out, act_in = two_range(et), two_range(psc)
        elif dve_lo == 0:
            act_out, act_in = et[:, dve_hi:4 * MCH], psc[:, dve_hi:4 * MCH]
        else:
            act_out, act_in = et[:, 0:dve_lo], psc[:, 0:dve_lo]
        nc.scalar.activation(
            out=act_out, in_=act_in,
            func=mybir.ActivationFunctionType.Exp,
            bias=0.0, scale=INV_SCALE,
        )
        nc.vector.tensor_scalar(
            out=et.bitcast(mybir.dt.int16)[:, dve_lo:dve_hi],
            in0=psc[:, dve_lo:dve_hi],
            scalar1=SCHRAUD_A, scalar2=SCHRAUD_B,
            op0=mybir.AluOpType.mult, op1=mybir.AluOpType.add,
        )
        ets[g] = et

    etps = {}

    def emit_pair_add_g(g):
        # DVE pair-add for sums; emitted BEFORE the next exp on the DVE queue
        # (its inputs are already complete) so the PE's sums matmuls don't
        # stall behind a Schraudolph exp still in the DVE FIFO.  (Offloading
        # alternate pairs to GPSIMD measured +3us -- SBUF-port contention.)
        if not (0 <= g < G) or (g % NB) % 2 != 1:
            return
        etp = sump.tile([128, 4 * MCH], BF16, tag="etp", name="etp")
        nc.vector.tensor_add(out=etp, in0=ets[g - 1], in1=ets[g])
        etps[g] = etp

    def emit_pv_g(g):
        if not (0 <= g < G):
            return
        mc, j = divmod(g, NB)
        if j == 0:
            pos[mc] = psum_o.tile([128, 2 * MCH], F32, tag="po", name="po")
        emit_pv(pos[mc], j, ets[g])
        ets.pop(g)

    def emit_sums_g(g):
        # Lagged one step behind PV so the DVE pair-add is always 2 steps old
        # by the time the PE reaches the sums matmuls (they stalled ~0.4us/g
        # waiting on the DVE queue when emitted alongside PV).  NOTE: any
        # GPSIMD tensor_add in this loop (half-pairs, or a level-2 quad add
        # even at 4-step lag) measured +3 to +34us -- do not offload there.
        if not (0 <= g < G) or (g % NB) % 2 != 1:
            return
        mc, j = divmod(g, NB)
        po = pos[mc]
        emit_sums(po, etps.pop(g), last=(j == NB - 1))
        if j == NB - 1:
            po = pos.pop(mc)
            rc = outp.tile([128, MCH], F32, tag="recip")
            nc.vector.reciprocal_approx_fast(out=rc, in_=po[:, MCH:2 * MCH])
            of = outp.tile([128, MCH], F32, tag="outf")
            nc.vector.tensor_mul(out=of, in0=po[:, 0:MCH], in1=rc)
            m0 = mc * MCH
            nc.sync.dma_start(out=out_d.ap()[:, m0:m0 + MCH], in_=of)

    # Interleave projection-chunk emission with the attention pipeline:
    # scores for n-block j only need x/kz chunk j//4 (+~2 n-blocks of
    # lookahead), so later chunks project while the first n-blocks' exps
    # already stream on ACT/DVE.
    next_chunk = 0

    def ensure_chunks_for_scores(gs):
        nonlocal next_chunk
        if not (0 <= gs < G):
            need = len(PROJ_CHUNKS)  # pipeline done; flush any stragglers
        else:
            j = gs % NB
            need = CHUNK_FOR_J[min(j + 2, NB - 1)] + 1 if gs < NB else len(PROJ_CHUNKS)
        while next_chunk < need:
            emit_proj_chunk(*PROJ_CHUNKS[next_chunk])
            next_chunk += 1

    # PE consumer work (PV/sums) batches on odd steps only: the PE stream
    # becomes [SC][SC, PV, PV, SUM] per 2 steps, halving the row<->col
    # tiling-config switches (each costs ~120ns of LDW serialization --
    # cross-mode LDWs cannot overlap running streams).  psc allocation
    # timing is unchanged, so PSUM pressure is identical.  (Batching 4 steps
    # measured SLOWER -- array occupancy dropped to 94%.)
    for g in range(-SLAG, G + 2):
        ensure_chunks_for_scores(g + SLAG)
        emit_scores_g(g + SLAG)
        emit_pair_add_g(g)
        emit_exp_g(g + 1)
        if g % 2 == 1:
            emit_pv_g(g - 2)
            emit_pv_g(g - 1)
            emit_sums_g(g - 2)
        pscs.pop(g, None)


def build_module() -> bass.Bass:
    nc = bacc.Bacc("TRN2", target_bir_lowering=False)
    xb_d = nc.declare_dram_parameter("xb", [128, 2 * M], BF16, isOutput=False)
    wb_d = nc.declare_dram_parameter("wb", [128, 3, 2, CH], BF16, isOutput=False)
    rh_d = nc.declare_dram_parameter("rh2", [64, 2 * M], F32, isOutput=False)
    out_d = nc.declare_dram_parameter("out", [CH, M], F32, isOutput=True)
    with tile.TileContext(nc) as tc, ExitStack() as ctx:
        _kernel_body(ctx, tc, xb_d, wb_d, rh_d, out_d)
    nc.compile()
    return nc


def pos_encoding_2d(c, h, w):
    """numpy port of the reference's fixed 2D sinusoidal PE -> (c, h*w)."""
    ch = c // 2
    div = np.float32(10000.0) ** (np.arange(0, ch, 2, dtype=np.float32) / np.float32(ch))
    py = np.arange(h, dtype=np.float32)[None, :] / div[:, None]
    px = np.arange(w, dtype=np.float32)[None, :] / div[:, None]
    pe_y = np.stack([np.sin(py), np.cos(py)], axis=1).reshape(ch, h).astype(np.float32)
    pe_x = np.stack([np.sin(px), np.cos(px)], axis=1).reshape(ch, w).astype(np.float32)
    pe = np.concatenate(
        [
            np.broadcast_to(pe_y[:, :, None], (ch, h, w)),
            np.broadcast_to(pe_x[:, None, :], (ch, h, w)),
        ],
        axis=0,
    )
    return np.ascontiguousarray(pe.reshape(c, h * w), dtype=np.float32)


_CACHE = {}


def _get_nc() -> bass.Bass:
    if "nc" not in _CACHE:
        _CACHE["nc"] = build_module()
    return _CACHE["nc"]


def make_in_maps(x, w_v, w_z, w_q):
    rh_full = pos_encoding_2d(C_OUT, H, W)
    x = np.asarray(x, dtype=np.float32)
    w_v = np.asarray(w_v, dtype=np.float32)
    w_z = np.asarray(w_z, dtype=np.float32)
    w_q = np.asarray(w_q, dtype=np.float32)
    in_maps = []
    for core in range(8):
        b, hh = core // 2, core % 2
        c0 = CH * hh
        # x: xb[p, k*M + j] = x[b, k*128 + p, j]
        xx = x[b].reshape(2, 128, M)
        xb = np.ascontiguousarray(
            xx.transpose(1, 0, 2).reshape(128, 2 * M)
        ).astype(NP_BF16)
        # weights: wb[p, i, k, c] = w_i[c0 + c, k*128 + p]
        wb = np.zeros((128, 3, 2, CH), np.float32)
        for i, wm in enumerate((w_v, w_z, w_q)):
            wb[:, i] = wm[c0:c0 + CH, :].T.reshape(2, 128, CH).transpose(1, 0, 2)
        wb = wb.astype(NP_BF16)
        # rh2: [32*(h%2)+c, (h//2)*M + m] = rh[c0 + 32h + c, m]  (rows 0-63)
        rh2 = np.zeros((64, 2 * M), np.float32)
        rh_c = rh_full[c0:c0 + CH, :].reshape(4, DH, M)  # [h, c, m]
        for h in range(4):
            r0 = DH * (h % 2)
            cb = (h // 2) * M
            rh2[r0:r0 + DH, cb:cb + M] = rh_c[h]
        in_maps.append({"xb": xb, "wb": wb, "rh2": rh2})
    return in_maps


def assemble_output(results):
    out = np.empty((B, C_OUT, H, W), np.float32)
    for core in range(8):
        b, hh = core // 2, core % 2
        out[b, CH * hh:CH * hh + CH] = results[core]["out"].reshape(CH, H, W)
    return out


def kernel(x, w_v, w_z, w_q, _trace=False):
    nc = _get_nc()
    in_maps = make_in_maps(x, w_v, w_z, w_q)
    res = run_bass_kernel_spmd(nc, in_maps, core_ids=list(range(8)), trace=_trace)
    out = assemble_output(res.results)
    if _trace:
        kernel.last_results = res
    return out


# revision 41
# speedup vs baseline: 1.0229x; 1.0229x over previous
"""MultiHeadSelfAttention2D Trainium2 kernel (8-core SPMD, full I/O).

Problem: B=4, C_IN=C_OUT=256, HEADS=8, H=W=48 (m = 2304), fp32.
  vh, zh, qh = per-head 1x1-conv projections of x; rh = fixed 2D sin/cos PE.
  scores = vh^T (zh + rh)  (per b,h); attn = softmax(scores/sqrt(dh), axis=n)
  out = attn @ qh^T  -> (b, c_out, h, w)

Sharding: core = 2*b + head_half. Each core handles one batch image and 4
heads (=128 output channels). No cross-core communication.

Per-core pipeline (bf16 matmul path, fp32 psum accumulation; layouts chosen
so no on-chip transposes are needed):
  - x / weights ship from host ALREADY in bf16 (halves input DMA, removes
    all on-chip casts: the casts cost ~6.5us ACT + ~3us DVE in v1)
  - vh2/kz2 [64, 2*2304] bf16: head h at row-block 32*(h%2), col-block h//2.
    Two row blocks let two scores matmuls run concurrently on disjoint PE
    row-groups while writing different psum banks (same-bank concurrent
    sub-array writes are fatal on TRN2).
  - qhT[n, ch] bf16, nb-major [128, 18*128]
  - attention runs as ONE flat software-pipelined loop over g = (m-chunk,
    n-block): scores(g+2) | exp(g+1) emitted every step; PV(g-2), PV(g-1),
    sums(g-2) batch on ODD steps only, so the PE stream is
    [SC][SC, PV, PV, SUM] per 2 steps.  The deep PV lag keeps the in-order
    PE queue from ever reaching a PV matmul before its exp block, and the
    2-step batching halves the row<->col tiling-config switches (each
    ~120ns: cross-mode LDWEIGHTS cannot overlap running streams).  Per g:
      scoresT[n,m] 4 matmuls (2-way row-concurrent) -> psc [128, 1024]
      -> exp split by COLUMNS every block: ACT does ~680 cols (exact Exp),
         DVE does ~340 via the Schraudolph bit trick (one tensor_scalar:
         bf16_bits = int16(psc*A + B), ~+-3%).  The DVE window ROTATES over
         3 positions with g%3 (a fixed window makes its m-columns 100%
         Schraudolph -> 1.5e-3 abs err, over budget; rotation dilutes to
         1/3 per output column like the passing block-alternating scheme).
         When DVE gets the middle piece, ACT covers the two outer pieces
         with ONE 2-range strided AP (stride 680 x 2, inner 344).
      -> PV col-tiled 4-head matmuls (fully concurrent); row-sums via
         ones-matmul every OTHER n-block on a DVE-precomputed et(j)+et(j+1)
         pair (halves PE's sums matmuls)
  - projections are emitted chunk-major and interleaved into the first
    n-blocks of the attention loop (chunk c gates scores per CHUNK_FOR_J
    with ~2 n-blocks of lookahead), k=0/k=1 DMA halves adjacent
  - DVE reciprocal_approx_fast + multiply for the normalization, DMA out

Measured: 202.5us (prior session) -> 179.8us; rel err 1.756e-2 (gate 2e-2).
Steady state ~873ns/g: PE array ~98% busy (the pacer), DVE 91%, ACT 85%.
Dead ends (all MEASURED this session or earlier, do not retry):
zero-weight filler matmuls serialize (+90-100us); 512-wide score windows /
4-way scores / MCH=512 all need >=9 psum banks (8 exist); fp8 DoubleRow
gains nothing (matmuls here are rhs-column-stream-bound); ANY gpsimd
tensor_add in the loop (+3us as alternate pair-adds at 2-step lag, +34us
as level-2 quad adds at 4-step lag); 4-step PE batching drops array
occupancy to 94%; 4-way proj col-groups + SBUF relocation of v (+5-13us
net, compresses the proj phase and loses the HAM-warm window);
block-alternating exp 12:6 (bimodal 873/1114ns pace -- the 3-deep psc
pool cannot buffer across the alternation); fixed-window column-split exp
(4.4e-2 rel err).  HAM: PE un-throttles to 2.4 GHz only at ~88%+
STREAM-cycle duty over 3.4us; the loop sits at ~85% -> 1.2 GHz forever.
"""

import numpy as np
from contextlib import ExitStack

import ml_dtypes

import concourse.bass as bass
import concourse.bacc as bacc
import concourse.tile as tile
from concourse import mybir
from concourse.bass_utils import run_bass_kernel_spmd

F32 = mybir.dt.float32
BF16 = mybir.dt.bfloat16
NP_BF16 = ml_dtypes.bfloat16

B, C_IN, C_OUT, HEADS, H, W = 4, 256, 256, 8, 48, 48
M = H * W  # 2304
DH = C_OUT // HEADS  # 32
HPC = 4  # heads per core
CH = HPC * DH  # 128 channels per core
NB = M // 128  # 18 n-blocks of 128
MCH = 256  # m-chunk width
NMCH = M // MCH  # 9
INV_SCALE = float(1.0 / np.sqrt(np.float32(DH)))  # softmax temperature

# ACT (exact exp) and DVE (Schraudolph int16 bit-trick exp) split EVERY
# exp block by columns.  Alternating whole blocks between the engines (v2)
# made the per-g pace bimodal -- ACT-blocks ran at ACT's 1114ns, DVE-blocks
# at the PE's 873ns, averaging 1034 -- because the 3-deep psc pool cannot
# buffer enough slack to amortize the alternation.  A per-block column
# split equalizes every g at ~880ns on all three engines.
#
# CRITICAL: the DVE window must ROTATE across n-blocks (g%3).  The et
# columns are the m axis; a FIXED window makes its output columns 100%
# Schraudolph (no dilution across the softmax's n-sum) -- measured abs err
# 1.5e-3, over the 7e-4 budget.  Rotating over 3 positions gives every
# output column exactly 1/3 Schraudolph n-contributions, same dilution as
# the passing v2/v3 configs (5.7e-4).  When the rotation puts DVE in the
# middle, ACT covers {[0,344) u [680,1024)} with ONE 2-range strided AP
# (stride 680 x 2, inner 344) -- two ACTIVATEs would cost +293ns/g.
EXP_SPLITS = [
    # (dve_lo, dve_hi, act is 2-range?)
    (0, 344, False),     # ACT [344, 1024)
    (344, 680, True),    # ACT [0,344) u [680,1024)
    (680, 1024, False),  # ACT [0, 680)
]
SCHRAUD_A = float(128.0 / np.log(2.0) * INV_SCALE)
# 127*128 (bf16 bias) - 5.51 (minimax centering) + 0.5 (trunc->round)
SCHRAUD_B = 16251.0

PROJ_CHUNKS = [(0, 512), (512, 512), (1024, 512), (1536, 512), (2048, 256)]
# chunk index that covers scores n-block j (kz2 cols j*128)
CHUNK_FOR_J = [
    next(ci for ci, (off, wd) in enumerate(PROJ_CHUNKS) if off <= j * 128 < off + wd)
    for j in range(NB)
]

# scores psum / exp-tile column block per head: two-way row concurrency,
# h0/h2 share psum bank 0 (row block 0), h1/h3 share bank 1 (row block 32)
ECOL = [0, 2, 1, 3]  # head -> 256-col block in psc/et


def _kernel_body(ctx: ExitStack, tc: tile.TileContext, xb_d, wb_d, rh_d, out_d):
    nc = tc.nc

    consts = ctx.enter_context(tc.tile_pool(name="consts", bufs=1))
    expp = ctx.enter_context(tc.tile_pool(name="expp", bufs=6))
    sump = ctx.enter_context(tc.tile_pool(name="sump", bufs=3))
    outp = ctx.enter_context(tc.tile_pool(name="outp", bufs=3))
    psum_s = ctx.enter_context(tc.tile_pool(name="psum_s", bufs=3, space="PSUM"))
    psum_o = ctx.enter_context(tc.tile_pool(name="psum_o", bufs=2, space="PSUM"))

    # ---- persistent SBUF tensors ----
    rh_sb = consts.tile([64, 2 * M], F32)
    x_bf = consts.tile([128, 2, M], BF16)
    wb_sb = consts.tile([128, 3, 2, CH], BF16)
    vh2 = consts.tile([64, 2 * M], BF16)  # [32*(h%2)+c, (h//2)*M + m]
    kz2 = consts.tile([64, 2 * M], BF16)
    qt_sb = consts.tile([128, M], BF16)   # [n, ch] nb-major blocks
    ones_sb = consts.tile([128, DH], BF16)

    wv_bf = wb_sb[:, 0]
    wz_bf = wb_sb[:, 1]
    wq_bf = wb_sb[:, 2]

    # wv/wz land first (wq is only needed by the qt projections, a few us
    # later), then x/rh interleaved per chunk with both contraction halves
    # (k=0,1) adjacent, so projection chunk c can start as soon as its slice
    # arrives instead of after most of the input.
    nc.sync.dma_start(out=wb_sb[:, 0:2], in_=wb_d.ap()[:, 0:2])
    for ci, (off, wd) in enumerate(PROJ_CHUNKS):
        for k in range(2):
            nc.sync.dma_start(
                out=x_bf[:, k, off:off + wd],
                in_=xb_d.ap()[:, k * M + off: k * M + off + wd],
            )
        if ci == 0:
            nc.sync.dma_start(out=wb_sb[:, 2:3], in_=wb_d.ap()[:, 2:3])
        for pair in range(2):
            nc.sync.dma_start(
                out=rh_sb[:, pair * M + off: pair * M + off + wd],
                in_=rh_d.ap()[:, pair * M + off: pair * M + off + wd],
            )
    nc.vector.memset(ones_sb, 1.0)

    # zero bias for Exp (one-time; cross-engine edge is paid once)

    # ---- projections (chunk-major; emission interleaved with attention) ----
    # 4-way col-group concurrency: z -> rows 32*(h%2) (q0/q32, in place for
    # kz2), v -> rows 64+32*(h%2) (q64/q96, staged then DMA'd down).  Both
    # pairs go in ONE psum tile (pair = bank); 4 concurrent streams per bank
    # at disjoint partitions are legal (only same-partition same-bank
    # concurrent writes are fatal).  Halves the vh/kz PE time vs the 2-way
    # v1 layout (z serialized behind v on the same col-groups there).
    def emit_proj_chunk(off, wd):
        # 2-way: v/z for heads (2*pair, 2*pair+1) at rows 32*(h%2); v in
        # bank 0, z in bank 1.  (A 4-way variant with z/v split across row
        # halves measured SLOWER overall: the needed SBUF->SBUF relocation
        # of v and the compressed proj phase cost more than the PE saved.)
        for pair in range(2):
            ps = psum_s.tile([128, 4 * MCH], F32, tag="psc", name="ps")
            for hh in range(2):  # head = 2*pair + hh
                h = 2 * pair + hh
                for k in range(2):
                    nc.tensor.matmul(
                        ps[32 * hh:32 * hh + 32, :wd],
                        lhsT=wv_bf[:, k, 32 * h:32 * h + 32],
                        rhs=x_bf[:, k, off:off + wd],
                        start=(k == 0),
                        stop=(k == 1),
                        tile_position=(0, 32 * hh),
                        skip_group_check=True,
                    )
                    nc.tensor.matmul(
                        ps[32 * hh:32 * hh + 32, 512:512 + wd],
                        lhsT=wz_bf[:, k, 32 * h:32 * h + 32],
                        rhs=x_bf[:, k, off:off + wd],
                        start=(k == 0),
                        stop=(k == 1),
                        tile_position=(0, 32 * hh),
                        skip_group_check=True,
                    )
            # vh2 copy on ACT (idle during the ramp) so DVE can focus on the
            # kz2 adds + early exps; kz2 gates the first scores matmuls
            nc.scalar.copy(
                out=vh2[:, pair * M + off: pair * M + off + wd], in_=ps[0:64, :wd]
            )
            nc.vector.tensor_add(
                out=kz2[:, pair * M + off: pair * M + off + wd],
                in0=ps[0:64, 512:512 + wd],
                in1=rh_sb[:, pair * M + off: pair * M + off + wd],
            )
        emit_proj_qt(off, wd)

    def emit_proj_qt(off, wd):
        # qhT for the n-blocks covered by this chunk, all in ONE psum tile
        # and one copy: fewer psum-pool rotations against the attention psc
        # stream
        nbs = list(range(off // 128, (off + wd) // 128))
        ps = psum_s.tile([128, 4 * MCH], F32, tag="psc", name="ps")
        for i, nb in enumerate(nbs):
            for k in range(2):
                nc.tensor.matmul(
                    ps[:, i * CH:(i + 1) * CH],
                    lhsT=x_bf[:, k, nb * 128:(nb + 1) * 128],
                    rhs=wq_bf[:, k, :],
                    start=(k == 0),
                    stop=(k == 1),
                )
        nc.vector.tensor_copy(
            out=qt_sb[:, nbs[0] * 128:(nbs[-1] + 1) * 128],
            in_=ps[:, 0:len(nbs) * CH],
        )

    # ---- attention ----
    def emit_scores(psc, j, m0):
        # h0 (rows 0-31 -> bank0) || h1 (rows 32-63 -> bank1) concurrent;
        # h2/h3 reuse the same PE rows so they serialize behind h0/h1.
        for h in (0, 1, 2, 3):
            rb = 32 * (h % 2)
            cb = (h // 2) * M
            nc.tensor.matmul(
                psc[:, ECOL[h] * MCH:(ECOL[h] + 1) * MCH],
                lhsT=kz2[rb:rb + 32, cb + j * 128: cb + (j + 1) * 128],
                rhs=vh2[rb:rb + 32, cb + m0: cb + m0 + MCH],
                start=True,
                stop=True,
                tile_position=(rb, 0),
                skip_group_check=True,
            )

    def emit_pv(po, j, et):
        # PV: out_unnorm[32h+d, m] += sum_n qhT[n, 32h+d] * expT_h[n, m]
        for h in range(HPC):
            nc.tensor.matmul(
                po[32 * h:32 * h + 32, 0:MCH],
                lhsT=qt_sb[:, j * 128 + 32 * h: j * 128 + 32 * h + 32],
                rhs=et[:, ECOL[h] * MCH:(ECOL[h] + 1) * MCH],
                start=(j == 0),
                stop=False,
                tile_position=(0, 32 * h),
                skip_group_check=True,
            )

    def emit_sums(po, etp, last):
        # row-sums of an et(j)+et(j+1) pair, replicated over each head slot
        for h in range(HPC):
            nc.tensor.matmul(
                po[32 * h:32 * h + 32, MCH:2 * MCH],
                lhsT=ones_sb,
                rhs=etp[:, ECOL[h] * MCH:(ECOL[h] + 1) * MCH],
                start=False,
                stop=last,
                tile_position=(0, 32 * h),
                skip_group_check=True,
            )

    # Flat software-pipelined loop over g = mc*NB + j. Scores run SLAG steps
    # ahead of exp, which runs one ahead of pair-adds, which run one ahead of
    # PV/sums -- the extra PV lag keeps the exp latency entirely off the PE
    # queue's critical path.
    SLAG = 2  # scores stream leads the exp stream by this many steps
    G = NMCH * NB
    pscs, ets, pos = {}, {}, {}

    def emit_scores_g(g):
        if not (0 <= g < G):
            return
        mc, j = divmod(g, NB)
        pscs[g] = psum_s.tile([128, 4 * MCH], F32, tag="psc", name="psc")
        emit_scores(pscs[g], j, mc * MCH)

    def two_range(ap_full):
        # {[0,344) u [680,1024)} as one AP: {stride 680 x 2} x {1 x 344}
        base = ap_full[:, 0:4 * MCH]
        dims = list(base.ap)
        return bass.AP(base.tensor, base.offset, [dims[0], [680, 2], [1, 344]])

    def emit_exp_g(g):
        if not (0 <= g < G):
            return
        psc = pscs[g]
        et = expp.tile([128, 4 * MCH], BF16, tag="et")
        dve_lo, dve_hi, act_split = EXP_SPLITS[g % 3]
        if act_split:
            act_out, act_in = two_range(et), two_range(psc)
        elif dve_lo == 0:
            act_out, act_in = et[:, dve_hi:4 * MCH], psc[:, dve_hi:4 * MCH]
        else:
            act_out, act_in = et[:, 0:dve_lo], psc[:, 0:dve_lo]
        nc.scalar.activation(
            out=act_out, in_=act_in,
            func=mybir.ActivationFunctionType.Exp,
            bias=0.0, scale=INV_SCALE,
        )
        nc.vector.tensor_scalar(
            out=et.bitcast(mybir.dt.int16)[:, dve_lo:dve_hi],
            in0=psc[:, dve_lo:dve_hi],
            scalar1=SCHRAUD_A, scalar2=SCHRAUD_B,
            op0=mybir.AluOpType.mult, op1=mybir.AluOpType.add,
        )
        ets[g] = et

    etps = {}

    def emit_pair_add_g(g):
        # DVE pair-add for sums; emitted BEFORE the next exp on the DVE queue
        # (its inputs are already complete) so the PE's sums matmuls don't
        # stall behind a Schraudolph exp still in the DVE FIFO.  (Offloading
        # alternate pairs to GPSIMD measured +3us -- SBUF-port contention.)
        if not (0 <= g < G) or (g % NB) % 2 != 1:
            return
        etp = sump.tile([128, 4 * MCH], BF16, tag="etp", name="etp")
        nc.vector.tensor_add(out=etp, in0=ets[g - 1], in1=ets[g])
        etps[g] = etp

    def emit_pv_g(g):
        if not (0 <= g < G):
            return
        mc, j = divmod(g, NB)
        if j == 0:
            pos[mc] = psum_o.tile([128, 2 * MCH], F32, tag="po", name="po")
        emit_pv(pos[mc], j, ets[g])
        ets.pop(g)

    def emit_sums_g(g):
        # Lagged one step behind PV so the DVE pair-add is always 2 steps old
        # by the time the PE reaches the sums matmuls (they stalled ~0.4us/g
        # waiting on the DVE queue when emitted alongside PV).  NOTE: any
        # GPSIMD tensor_add in this loop (half-pairs, or a level-2 quad add
        # even at 4-step lag) measured +3 to +34us -- do not offload there.
        if not (0 <= g < G) or (g % NB) % 2 != 1:
            return
        mc, j = divmod(g, NB)
        po = pos[mc]
        emit_sums(po, etps.pop(g), last=(j == NB - 1))
        if j == NB - 1:
            po = pos.pop(mc)
            rc = outp.tile([128, MCH], F32, tag="recip")
            nc.vector.reciprocal_approx_fast(out=rc, in_=po[:, MCH:2 * MCH])
            of = outp.tile([128, MCH], F32, tag="outf")
            nc.vector.tensor_mul(out=of, in0=po[:, 0:MCH], in1=rc)
            m0 = mc * MCH
            nc.sync.dma_start(out=out_d.ap()[:, m0:m0 + MCH], in_=of)

    # Interleave projection-chunk emission with the attention pipeline:
    # scores for n-block j only need x/kz chunk j//4 (+~2 n-blocks of
    # lookahead), so later chunks project while the first n-blocks' exps
    # already stream on ACT/DVE.
    next_chunk = 0

    def ensure_chunks_for_scores(gs):
        nonlocal next_chunk
        if not (0 <= gs < G):
            need = len(PROJ_CHUNKS)  # pipeline done; flush any stragglers
        else:
            j = gs % NB
            need = CHUNK_FOR_J[min(j + 2, NB - 1)] + 1 if gs < NB else len(PROJ_CHUNKS)
        while next_chunk < need:
            emit_proj_chunk(*PROJ_CHUNKS[next_chunk])
            next_chunk += 1

    # PE consumer work (PV/sums) batches on odd steps only: the PE stream
    # becomes [SC][SC, PV, PV, SUM] per 2 steps, halving the row<->col
    # tiling-config switches (each costs ~120ns of LDW serialization --
    # cross-mode LDWs cannot overlap running streams).  psc allocation
    # timing is unchanged, so PSUM pressure is identical.  (Batching 4 steps
    # measured SLOWER -- array occupancy dropped to 94%.)
    for g in range(-SLAG, G + 2):
        ensure_chunks_for_scores(g + SLAG)
        emit_scores_g(g + SLAG)
        emit_pair_add_g(g)
        emit_exp_g(g + 1)
        if g % 2 == 1:
            emit_pv_g(g - 2)
            emit_pv_g(g - 1)
            emit_sums_g(g - 2)
        pscs.pop(g, None)


def build_module() -> bass.Bass:
    nc = bacc.Bacc("TRN2", target_bir_lowering=False)
    xb_d = nc.declare_dram_parameter("xb", [128, 2 * M], BF16, isOutput=False)
    wb_d = nc.declare_dram_parameter("wb", [128, 3, 2, CH], BF16, isOutput=False)
    rh_d = nc.declare_dram_parameter("rh2", [64, 2 * M], F32, isOutput=False)
    out_d = nc.declare_dram_parameter("out", [CH, M], F32, isOutput=True)
    with tile.TileContext(nc) as tc, ExitStack() as ctx:
        _kernel_body(ctx, tc, xb_d, wb_d, rh_d, out_d)
    nc.compile()
    return nc


def pos_encoding_2d(c, h, w):
    """numpy port of the reference's fixed 2D sinusoidal PE -> (c, h*w)."""
    ch = c // 2
    div = np.float32(10000.0) ** (np.arange(0, ch, 2, dtype=np.float32) / np.float32(ch))
    py = np.arange(h, dtype=np.float32)[None, :] / div[:, None]
    px = np.arange(w, dtype=np.float32)[None, :] / div[:, None]
    pe_y = np.stack([np.sin(py), np.cos(py)], axis=1).reshape(ch, h).astype(np.float32)
    pe_x = np.stack([np.sin(px), np.cos(px)], axis=1).reshape(ch, w).astype(np.float32)
    pe = np.concatenate(
        [
            np.broadcast_to(pe_y[:, :, None], (ch, h, w)),
            np.broadcast_to(pe_x[:, None, :], (ch, h, w)),
        ],
        axis=0,
    )
    return np.ascontiguousarray(pe.reshape(c, h * w), dtype=np.float32)


_CACHE = {}


def _get_nc() -> bass.Bass:
    if "nc" not in _CACHE:
        _CACHE["nc"] = build_module()
    return _CACHE["nc"]


def make_in_maps(x, w_v, w_z, w_q):
    rh_full = pos_encoding_2d(C_OUT, H, W)
    x = np.asarray(x, dtype=np.float32)
    w_v = np.asarray(w_v, dtype=np.float32)
    w_z = np.asarray(w_z, dtype=np.float32)
    w_q = np.asarray(w_q, dtype=np.float32)
    in_maps = []
    for core in range(8):
        b, hh = core // 2, core % 2
        c0 = CH * hh
        # x: xb[p, k*M + j] = x[b, k*128 + p, j]
        xx = x[b].reshape(2, 128, M)
        xb = np.ascontiguousarray(
            xx.transpose(1, 0, 2).reshape(128, 2 * M)
        ).astype(NP_BF16)
        # weights: wb[p, i, k, c] = w_i[c0 + c, k*128 + p]
        wb = np.zeros((128, 3, 2, CH), np.float32)
        for i, wm in enumerate((w_v, w_z, w_q)):
            wb[:, i] = wm[c0:c0 + CH, :].T.reshape(2, 128, CH).transpose(1, 0, 2)
        wb = wb.astype(NP_BF16)
        # rh2: [32*(h%2)+c, (h//2)*M + m] = rh[c0 + 32h + c, m]  (rows 0-63)
        rh2 = np.zeros((64, 2 * M), np.float32)
        rh_c = rh_full[c0:c0 + CH, :].reshape(4, DH, M)  # [h, c, m]
        for h in range(4):
            r0 = DH * (h % 2)
            cb = (h // 2) * M
            rh2[r0:r0 + DH, cb:cb + M] = rh_c[h]
        in_maps.append({"xb": xb, "wb": wb, "rh2": rh2})
    return in_maps


def assemble_output(results):
    out = np.empty((B, C_OUT, H, W), np.float32)
    for core in range(8):
        b, hh = core // 2, core % 2
        out[b, CH * hh:CH * hh + CH] = results[core]["out"].reshape(CH, H, W)
    return out


def kernel(x, w_v, w_z, w_q, _trace=False):
    nc = _get_nc()
    in_maps = make_in_maps(x, w_v, w_z, w_q)
    res = run_bass_kernel_spmd(nc, in_maps, core_ids=list(range(8)), trace=_trace)
    out = assemble_output(res.results)
    if _trace:
        kernel.last_results = res
    return out
